# revision 40
# baseline (speedup 1.0000x reference)
"""Trainium2 Bass kernel for nn_BalancedLoss (composite segmentation loss).

Pure data-parallel over 8 NeuronCores (2 samples each); each core emits a
[128, NQ*NWIN] tile of partial reductions which the host combines in fp64
(including the global min/max normalization algebra for gradient_magnitude).

v2 design vs baseline (915 us):
  - sigmoid replaced by th = tanh(p/2) (activation-table set 0); host algebra
    expands pp = (th+1)/2 through every product sum.
  - softplus(p) = ln2 - ln(1 - th) reuses th; one ACT op (set 5, batched
    per 2 windows with the sqrt block to amortize table loads).
  - curvature sigmoid(10*tanh(0.1*lp)) ~= sigmoid(lp) = (tanh(lp/2)+1)/2:
    one ACT op instead of two (error ~1e-5 of total, tol 2e-2).
  - dem mean/std computed on host (removes the on-device stats prepass).
  - erode threshold fused into the s2 product: (er > 8.5) * p in one DVE op.
  - pred-side Sobel PSUM evacuated by ACT Square (free accum of sum(gx^2));
    dem-side by DVE evac + tensor_tensor_reduce square (free accum).
  - g^2 assembly fused with min/max via tensor_tensor_reduce.
  - engine split so DVE/ACT/Pool all carry ~10 us/window; PE ~9.5.
  - per-window SP observer nop chain: every engine's first touch of the
    window's DMA'd tiles waits one SP sem; stale cross-engine WAR waits are
    stripped in the sync-minimizer using per-window sem-count snapshots.
"""

import os
import numpy as np
from contextlib import ExitStack

DEBUG_NWIN = int(os.environ.get("KDBG_NWIN", "9"))

B, H, W = 16, 1024, 1024
NCORES = 8
SPC = B // NCORES  # samples per core
EPS = 1e-8
NPIX = H * W
NTOT = B * NPIX

WINDOWS = [(0, 0, 125)] + [(122 * w, 3, 125) for w in range(1, 8)] + [(896, 83, 128)]
NW_PER_SAMPLE = len(WINDOWS)
NWIN = SPC * NW_PER_SAMPLE

# accumulator quantity indices, grouped by writer engine:
# ACT block [0..6)
Q_TH, Q_LN, Q_HN, Q_TH2, Q_SA, Q_SB = range(6)
# DVE block [6..17)
(Q_MINP, Q_MAXP, Q_MIND, Q_MAXD, Q_EP, Q_TP, Q_THTH2, Q_THHN,
 Q_AB, Q_G2P, Q_G2D) = range(6, 17)
NQ = 17
NQ_ACT, NQ_DVE = 6, 11

FBIG = 3.0e38


def _tridiag(a, b, c, dtype):
    # out[p] = a*x[p-1] + b*x[p] + c*x[p+1] for matmul out = lhsT.T @ x
    M = np.zeros((128, 128), dtype=np.float64)
    idx = np.arange(128)
    M[idx, idx] = b
    M[idx[:-1], idx[1:]] = a
    M[idx[1:], idx[:-1]] = c
    return M.astype(dtype)


def _build_consts():
    import ml_dtypes
    bf16 = ml_dtypes.bfloat16
    mats = [
        _tridiag(1, 1, 1, bf16),     # 0 M111   (box sum / dilate / erode)
        _tridiag(1, 2, 1, bf16),     # 1 M121   (sobel x, +1 col)
        _tridiag(-1, -2, -1, bf16),  # 2 -M121  (sobel x, -1 col)
        _tridiag(-1, 0, 1, bf16),    # 3 Mm101  (sobel y, +-1 col)
        _tridiag(-2, 0, 2, bf16),    # 4 Mm202  (sobel y, center)
        _tridiag(1, -8, 1, bf16),    # 5 M1m81  (9*(box_mean - x) center tap)
        _tridiag(0, 1, 0, bf16),     # 6 I      (laplacian +-1 col)
        _tridiag(1, -4, 1, bf16),    # 7 M1m41  (laplacian center)
        np.zeros((128, 2), bf16),    # 8 zero pad source
    ]
    return np.concatenate(mats, axis=1)  # [128, 8*128+2]


_NC_CACHE = {}


def _build_nc():
    if "nc" in _NC_CACHE:
        return _NC_CACHE["nc"]
    import concourse.bass as bass
    import concourse.tile as tile
    from concourse import mybir

    fp32 = mybir.dt.float32
    bf16 = mybir.dt.bfloat16
    ALU = mybir.AluOpType
    ACTF = mybir.ActivationFunctionType

    nc = bass.Bass("TRN2", target_bir_lowering=False)
    pred_d = nc.declare_dram_parameter("pred", [SPC, H, W], fp32, isOutput=False)
    targ_d = nc.declare_dram_parameter("target", [SPC, H, W], fp32, isOutput=False)
    dem_d = nc.declare_dram_parameter("dem", [SPC, H, W], fp32, isOutput=False)
    cbf16_d = nc.declare_dram_parameter("cbf16", [128, 8 * 128 + 2], bf16,
                                        isOutput=False)
    stats_d = nc.declare_dram_parameter("stats", [128, 2 * SPC], fp32,
                                        isOutput=False)
    out_d = nc.declare_dram_parameter("out", [128, NQ * NWIN], fp32, isOutput=True)

    # instruction markers for the window-snapshot strip rule
    win_marks = []   # per gw: dict(absorb=ins, obs=ins)
    strip_info = {"marks": win_marks}

    with tile.TileContext(nc) as tc:
        ctx = ExitStack()
        const = ctx.enter_context(tc.tile_pool(name="const", bufs=1))
        accp = ctx.enter_context(tc.tile_pool(name="accp", bufs=1))
        scr = ctx.enter_context(tc.tile_pool(name="scr", bufs=2))
        dfr = ctx.enter_context(tc.tile_pool(name="dfr", bufs=4))
        psum_v = ctx.enter_context(tc.tile_pool(name="psum_v", bufs=2, space="PSUM"))
        psum_a = ctx.enter_context(tc.tile_pool(name="psum_a", bufs=2, space="PSUM"))

        CB = const.tile([128, 8 * 128 + 2], bf16)

        def dma2(out_ap, in_ap, after=None):
            a = nc.sync.dma_start(out=out_ap[0:64], in_=in_ap[0:64])
            b = nc.sync.dma_start(out=out_ap[64:128], in_=in_ap[64:128])
            if after is not None:
                tile.add_dep_helper(a.ins, after.ins, sync=False,
                                    reason="order after absorber")
                tile.add_dep_helper(b.ins, after.ins, sync=False,
                                    reason="order after absorber")
            return [a, b]

        startup_dmas = []
        startup_dmas += dma2(CB, cbf16_d[:, :])
        ST = const.tile([128, 2 * SPC], fp32)
        startup_dmas.append(nc.sync.dma_start(out=ST, in_=stats_d[:, :]))
        ZPAD = cbf16_d[:, 8 * 128:8 * 128 + 2]

        def mb(i):
            return CB[:, i * 128:(i + 1) * 128]

        M111B, M121B, M121NB, M101B, M202B, M1M81B, IB, MLAPB = (
            mb(0), mb(1), mb(2), mb(3), mb(4), mb(5), mb(6), mb(7))

        # persistent rotating tiles (Tpp 4-deep for 4-window ACT batching)
        DBL = {}
        for par in (0, 1):
            DBL[("Tt", par)] = const.tile([128, 1024], fp32, name=f"Tt{par}")
            DBL[("Tp", par)] = const.tile([128, 1024], fp32, name=f"Tp{par}")
            DBL[("Td", par)] = const.tile([128, 1024], fp32, name=f"Td{par}")
            for nm in ("Ttb", "Tdb", "Te", "Tdl"):
                DBL[(nm, par)] = const.tile([128, 1026], bf16, name=f"{nm}{par}")
        for par in range(4):
            DBL[("Tpp", par)] = const.tile([128, 1026], bf16, name=f"Tpp{par}")

        # zero the pad columns of all padded tiles via DMA from the zero
        # columns of the const tensor (bf16 memset is not encodable)
        for par in (0, 1):
            for nm in ("Ttb", "Tdb", "Te", "Tdl"):
                t = DBL[(nm, par)]
                startup_dmas += dma2(t[:, 0:1026:1025], ZPAD)
        for par in range(4):
            startup_dmas += dma2(DBL[("Tpp", par)][:, 0:1026:1025], ZPAD)
        # eps bias tiles (fp32 memsets encode fine)
        EPS4B = const.tile([128, 1], fp32)
        nc.gpsimd.memset(EPS4B, 4.0 * EPS)
        EPS1B = const.tile([128, 1], fp32)
        eps_memset = nc.gpsimd.memset(EPS1B, EPS)
        # windows 0/1's obs chains wait all startup DMAs + the eps memsets,
        # so the cover-based strip handles every startup dependency.
        startup_last = startup_dmas + [eps_memset]

        # accumulators: ACT gets a side tile; DVE block lives in ACCBIG
        ACCBIG = accp.tile([128, NQ * NWIN], fp32, name="accbig")
        ACTACC = accp.tile([128, NQ_ACT * NWIN], fp32, name="actacc")

        def acc(q, wcol):
            if q < NQ_ACT:
                return ACTACC[:, q * NWIN + wcol:q * NWIN + wcol + 1]
            return ACCBIG[:, q * NWIN + wcol:q * NWIN + wcol + 1]

        def _all_instrs():
            return [i for b in nc.m.functions[0].blocks for i in b.instructions]

        def conv(ps, groups, srctile):
            # tiny touch matmuls absorb the psum-slot WAR wait so the first
            # real matmul carries only its producer wait (1-wait limit)
            for c0 in (0, 512):
                nc.tensor.matmul(ps[0:1, c0:c0 + 1], CB[:, 0:1], CB[:, 0:1],
                                 start=True, stop=True)
            last = None
            for c0 in (0, 512):
                for i, (mat, dx) in enumerate(groups):
                    last = nc.tensor.matmul(
                        ps[:, c0:c0 + 512], mat,
                        srctile[:, c0 + dx + 1:c0 + dx + 1 + 512],
                        start=(i == 0), stop=(i == len(groups) - 1))
            return last

        last_eng_op = {}   # gw -> {engine: ins} last op per engine per window
        deferred_act = []  # (kind, gw) ACT sqrt/ln ops batched per 2 windows

        nwin_run = SPC * min(DEBUG_NWIN, NW_PER_SAMPLE)
        for s in range(SPC):
            inv_ap = ST[:, 2 * s:2 * s + 1]
            nb_ap = ST[:, 2 * s + 1:2 * s + 2]
            for wi, (r0, p0, p1) in enumerate(WINDOWS[:DEBUG_NWIN]):
                gw = s * min(DEBUG_NWIN, NW_PER_SAMPLE) + wi
                wcol = s * NW_PER_SAMPLE + wi
                par = gw % 2
                Tt, Tp, Td = DBL[("Tt", par)], DBL[("Tp", par)], DBL[("Td", par)]
                Ttb, Tdb = DBL[("Ttb", par)], DBL[("Tdb", par)]
                Tpp = DBL[("Tpp", gw % 4)]
                Te, Tdl = DBL[("Te", par)], DBL[("Tdl", par)]

                win_i0 = len(_all_instrs())
                # ---- absorber: wait last op of each engine two windows ago.
                # NoOps have a single wait slot; chain 1-wait nops — the
                # in-order SP sequencer serializes the chain.
                prev = last_eng_op.get(gw - 2, {}) if gw >= 2 else {}
                absorb = None
                chain = None
                chain_ins = []
                for eng, ins in list(prev.items()) or [(None, None)]:
                    o = nc.sync.nop()
                    if ins is not None:
                        tile.add_dep_helper(o.ins, ins.ins, sync=True,
                                            reason=f"absorb w-2 {eng}")
                    if chain is not None:
                        tile.add_dep_helper(o.ins, chain.ins, sync=False,
                                            reason="absorb chain")
                    if absorb is None:
                        absorb = o
                    chain = o
                    chain_ins.append(o.ins)
                dmas = []
                dmas += dma2(Tt, targ_d[s, r0:r0 + 128, :], after=chain)
                dmas += dma2(Tp, pred_d[s, r0:r0 + 128, :], after=chain)
                dmas += dma2(Td, dem_d[s, r0:r0 + 128, :], after=chain)
                obs = None
                obs_deps = list(dmas)
                if gw < 2:
                    obs_deps += startup_last
                for dd in obs_deps:
                    o = nc.sync.nop()
                    tile.add_dep_helper(o.ins, dd.ins, sync=True,
                                        reason="window dma observer")
                    if obs is not None:
                        tile.add_dep_helper(o.ins, obs.ins, sync=False,
                                            reason="obs chain")
                    obs = o
                    chain_ins.append(o.ins)
                win_marks.append({"absorb": absorb.ins, "obs": obs.ins,
                                  "chain": chain_ins})
                leo = {}

                # ---- DVE: target convert first (gates PE bx)
                tb_i = nc.vector.tensor_scalar(
                    out=Ttb[:, 1:1025], in0=Tt, scalar1=1.0, scalar2=None,
                    op0=ALU.mult)
                tile.add_dep_helper(tb_i.ins, obs.ins, sync=True, reason="obs")

                # ---- ACT: th gates PE sobel-p; dem convert gates sobel-d
                th_i = nc.scalar.activation(
                    out=Tpp[:, 1:1025], in_=Tp, func=ACTF.Tanh, scale=0.5,
                    accum_out=acc(Q_TH, wcol))
                tile.add_dep_helper(th_i.ins, obs.ins, sync=True, reason="obs")
                if gw >= 1:
                    tile.add_dep_helper(th_i.ins,
                                        last_eng_op[gw - 1]["ACT"].ins,
                                        sync=False, reason="act order")
                db_i = nc.scalar.activation(
                    out=Tdb[:, 1:1025], in_=Td, func=ACTF.Copy)
                tile.add_dep_helper(db_i.ins, obs.ins, sync=True, reason="obs")

                # ---- PE: bx (edge detect of target)
                bx = psum_v.tile([128, 1024], fp32, tag="psv")
                conv(bx, [(M111B, -1), (M1M81B, 0), (M111B, 1)], Ttb)

                # ---- PE: sobel on pred prob (after th)
                gxp = psum_a.tile([128, 1024], fp32, tag="psa")
                conv(gxp, [(M121NB, -1), (M121B, 1)], Tpp)
                gyp = psum_a.tile([128, 1024], fp32, tag="psa")
                conv(gyp, [(M101B, -1), (M101B, 1), (M202B, 0)], Tpp)

                # ---- ACT+DVE: edge threshold (gates PE dl)
                # |bx| > 1.35 as bx^2 > 1.8225 (Square is a set-0 ACT op)
                bx2 = scr.tile([128, 1024], bf16, tag="bx2")
                nc.scalar.activation(out=bx2, in_=bx, func=ACTF.Square)
                e_i = nc.vector.tensor_scalar(
                    out=Te[:, 1:1025], in0=bx2, scalar1=1.8225, scalar2=None,
                    op0=ALU.is_gt)

                # ---- ACT: pred-side square evacuations (set0, accum)
                sq_px = scr.tile([128, 1024], bf16, tag="sq_px")
                nc.scalar.activation(out=sq_px, in_=gxp, func=ACTF.Square)
                sq_py = scr.tile([128, 1024], bf16, tag="sq_py")
                nc.scalar.activation(out=sq_py, in_=gyp, func=ACTF.Square)

                # ---- PE: dilate
                dl = psum_v.tile([128, 1024], fp32, tag="psv")
                conv(dl, [(M111B, -1), (M111B, 0), (M111B, 1)], Te)

                # ---- PE: sobel on dem
                gxd = psum_a.tile([128, 1024], fp32, tag="psa")
                conv(gxd, [(M121NB, -1), (M121B, 1)], Tdb)
                gyd = psum_a.tile([128, 1024], fp32, tag="psa")
                conv(gyd, [(M101B, -1), (M101B, 1), (M202B, 0)], Tdb)

                # ---- DVE: dilate threshold (gates PE er)
                nc.vector.tensor_scalar(
                    out=Tdl[:, 1:1025], in0=dl, scalar1=0.5, scalar2=None,
                    op0=ALU.is_gt)

                # ---- ACT: dem-side square evacuations (set0, accum)
                sq_dx = scr.tile([128, 1024], bf16, tag="sq_dx")
                nc.scalar.activation(out=sq_dx, in_=gxd, func=ACTF.Square)
                sq_dy = scr.tile([128, 1024], bf16, tag="sq_dy")
                nc.scalar.activation(out=sq_dy, in_=gyd, func=ACTF.Square)

                # ---- PE: erode
                er = psum_v.tile([128, 1024], fp32, tag="psv")
                conv(er, [(M111B, -1), (M111B, 0), (M111B, 1)], Tdl)

                # ---- DVE: g2 assembly (STT carries the sum) + min/max
                g2p = dfr.tile([128, 1024], bf16, tag="g2p")
                nc.vector.scalar_tensor_tensor(
                    out=g2p, in0=sq_px, scalar=1.0, in1=sq_py,
                    op0=ALU.mult, op1=ALU.add, accum_out=acc(Q_G2P, wcol))
                nc.vector.tensor_reduce(out=acc(Q_MINP, wcol), in_=g2p,
                                        axis=mybir.AxisListType.X, op=ALU.min)
                nc.vector.tensor_reduce(out=acc(Q_MAXP, wcol), in_=g2p,
                                        axis=mybir.AxisListType.X, op=ALU.max)
                g2d = dfr.tile([128, 1024], bf16, tag="g2d")
                nc.vector.scalar_tensor_tensor(
                    out=g2d, in0=sq_dx, scalar=1.0, in1=sq_dy,
                    op0=ALU.mult, op1=ALU.add, accum_out=acc(Q_G2D, wcol))
                nc.vector.tensor_reduce(out=acc(Q_MIND, wcol), in_=g2d,
                                        axis=mybir.AxisListType.X, op=ALU.min)
                nc.vector.tensor_reduce(out=acc(Q_MAXD, wcol), in_=g2d,
                                        axis=mybir.AxisListType.X, op=ALU.max)

                # ---- PE: laplacian
                lp = psum_v.tile([128, 1024], fp32, tag="psv")
                pe_last = conv(lp, [(IB, -1), (MLAPB, 0), (IB, 1)], Tdb)

                # ---- DVE: fused erode-threshold * pred accumulation
                jp = scr.tile([128, 1024], bf16, tag="junkD")
                s2_i = nc.vector.scalar_tensor_tensor(
                    out=jp, in0=er, scalar=8.5, in1=Tp,
                    op0=ALU.is_gt, op1=ALU.mult, accum_out=acc(Q_EP, wcol))

                # ---- ACT: z-score square + gaussian + curvature tanh
                z2 = scr.tile([128, 1024], bf16, tag="z2")
                nc.scalar.activation(out=z2, in_=Tdb[:, 1:1025],
                                     func=ACTF.Square, scale=inv_ap, bias=nb_ap)
                hn = scr.tile([128, 1024], bf16, tag="hn")
                nc.scalar.activation(out=hn, in_=z2, func=ACTF.Exp, scale=-0.5,
                                     accum_out=acc(Q_HN, wcol))
                cs = scr.tile([128, 1024], bf16, tag="cs")
                cs_i = nc.scalar.activation(out=cs, in_=lp, func=ACTF.Tanh,
                                            scale=0.5,
                                            accum_out=acc(Q_TH2, wcol))

                # ---- DVE: product sums
                j1 = scr.tile([128, 1024], bf16, tag="junkP")
                s1_i = nc.vector.scalar_tensor_tensor(
                    out=j1, in0=Ttb[:, 1:1025], scalar=1.0, in1=Tp,
                    op0=ALU.mult, op1=ALU.mult, accum_out=acc(Q_TP, wcol))
                j4 = scr.tile([128, 1024], bf16, tag="junkP")
                nc.vector.scalar_tensor_tensor(
                    out=j4, in0=Tpp[:, 1:1025], scalar=1.0, in1=cs,
                    op0=ALU.mult, op1=ALU.mult, accum_out=acc(Q_THTH2, wcol))
                j5 = scr.tile([128, 1024], bf16, tag="junkP")
                s5_i = nc.vector.scalar_tensor_tensor(
                    out=j5, in0=Tpp[:, 1:1025], scalar=1.0, in1=hn,
                    op0=ALU.mult, op1=ALU.mult, accum_out=acc(Q_THHN, wcol))

                # ---- deferred ACT blocks (sqrt set3, ln set5) every 2 windows
                deferred_act.append((gw, wcol, g2p, g2d, gw % 4))
                act_last = cs_i
                dve_extra = None
                if gw % 4 == 3 or gw == nwin_run - 1:
                    av_tiles = []
                    prev_a = cs_i
                    for (dgw, dwcol, dg2p, dg2d, dpar) in deferred_act:
                        av_p = dfr.tile([128, 1024], bf16, tag="av_p",
                                        name=f"av_p{dgw}")
                        a1 = nc.scalar.activation(out=av_p, in_=dg2p,
                                                  func=ACTF.Sqrt, bias=EPS4B,
                                                  accum_out=acc(Q_SA, dwcol))
                        tile.add_dep_helper(a1.ins, prev_a.ins, sync=False,
                                            reason="act order")
                        av_d = dfr.tile([128, 1024], bf16, tag="av_d",
                                        name=f"av_d{dgw}")
                        a2 = nc.scalar.activation(out=av_d, in_=dg2d,
                                                  func=ACTF.Sqrt, bias=EPS1B,
                                                  accum_out=acc(Q_SB, dwcol))
                        tile.add_dep_helper(a2.ins, a1.ins, sync=False,
                                            reason="act order")
                        prev_a = a2
                        av_tiles.append((dgw, dwcol, dpar, av_p, av_d))
                    for (dgw, dwcol, dpar, av_p, av_d) in av_tiles:
                        jl = dfr.tile([128, 1024], bf16, tag="junkA",
                                      name=f"jl{dgw}")
                        act_last = nc.scalar.activation(
                            out=jl, in_=DBL[("Tpp", dpar)][:, 1:1025],
                            func=ACTF.Ln, scale=-1.0, bias=1.0,
                            accum_out=acc(Q_LN, dwcol))
                        tile.add_dep_helper(act_last.ins, prev_a.ins,
                                            sync=False, reason="act order")
                        prev_a = act_last
                    # DVE: s3 = sum(av_p*av_d)
                    for (dgw, dwcol, dpar, av_p, av_d) in av_tiles:
                        j3 = dfr.tile([128, 1024], bf16, tag="junkQ",
                                      name=f"j3{dgw}")
                        dve_extra = nc.vector.scalar_tensor_tensor(
                            out=j3, in0=av_p, scalar=1.0, in1=av_d,
                            op0=ALU.mult, op1=ALU.mult,
                            accum_out=acc(Q_AB, dwcol))
                    deferred_act = []

                # last ops per engine for the absorber two windows ahead.
                # The scheduler may reorder within an engine stream, so pin
                # each designated last op behind all its window peers with
                # nosync (ordering-only) edges.
                leo["PE"] = pe_last
                leo["DVE"] = dve_extra if dve_extra is not None else s5_i
                leo["ACT"] = act_last
                new_ins = _all_instrs()[win_i0:]
                for eng_key, lo in leo.items():
                    sem = {"PE": "PE", "DVE": "DVE", "ACT": "Activation"}[eng_key]
                    for x in new_ins:
                        e = getattr(x, "engine", None)
                        if e is None or str(e).split(".")[-1] != sem:
                            continue
                        if x is lo.ins:
                            continue
                        tile.add_dep_helper(lo.ins, x, sync=False,
                                            reason="fence order")
                last_eng_op[gw] = leo

        # tail: mirror the ACT accum tile into ACCBIG via DVE, then store
        nc.vector.tensor_scalar(out=ACCBIG[:, 0:NQ_ACT * NWIN], in0=ACTACC,
                                scalar1=1.0, scalar2=None, op0=ALU.mult)
        nc.sync.dma_start(out=out_d[:, :], in_=ACCBIG[:, :])
        ctx.close()

    _minimize_syncs(nc, strip_info)
    _NC_CACHE["nc"] = nc
    return nc


def _minimize_syncs(nc, strip_info):
    """Walrus codegen wants few sync-waits per instruction. Tile's emission is
    per-instruction structural; prune:
      1. per-engine observed-threshold replay (monotone sems).
      2. window-snapshot strip: ops of window w may drop waits with
         thresholds <= the sem counts at the end of window w-2 (covered by
         the absorber -> dma -> obs chain) and DMA-queue waits <= counts at
         obs_w. Applies to compute engines only.
      3. same-engine self-sem waits (program order).
      4. PE pc-monotone cleanup, final-store and drain special cases.
    """
    marks = strip_info["marks"]
    absorb_ids = {id(m["absorb"]): k for k, m in enumerate(marks)}
    obs_ids = {id(m["obs"]): k for k, m in enumerate(marks)}

    ENG_SEM = {"PE": "PE", "DVE": "DVE", "Activation": "Activation",
               "Pool": "Pool", "SP": "SP_sequencer"}

    def eng_of(ins):
        e = getattr(ins, "engine", None)
        return str(e).split(".")[-1] if e is not None else "SP"

    blocks = nc.m.functions[0].blocks
    nonmono = set()
    for blk in blocks:
        for ins in blk.instructions:
            if ins.sync_info is None:
                continue
            for u in ins.sync_info.on_update:
                um = str(u.update_mode)
                if "sub" in um or "dec" in um:
                    nonmono.add(u.ant_name)

    # pass A: window attribution in EMISSION order (instruction-id order —
    # the scheduler reorders the block), and per-window wait coverage read
    # directly off the absorber/observer chain nops' framework-assigned
    # wait values (same value space as every other wait).
    def _ord(ins):
        nm = getattr(ins, "name", "")
        try:
            return int(str(nm).rsplit("-", 1)[-1])
        except ValueError:
            return 1 << 60
    all_ins = [ins for blk in blocks for ins in blk.instructions]
    all_ins.sort(key=_ord)
    cur_win = -1
    win_of_ins = {}
    for ins in all_ins:
        iid = id(ins)
        win_of_ins[iid] = cur_win
        if iid in obs_ids:
            k = obs_ids[iid]
            cur_win = k
            win_of_ins[iid] = k
    # coverage: cover[k][sem] = max wait value guaranteed satisfied once
    # window k's obs chain has retired (accumulates across windows via the
    # in-order SP sequencer).
    cover = []
    acc_cov = {}
    for k, m in enumerate(marks):
        for cins in m["chain"]:
            si = cins.sync_info
            if si is None:
                continue
            for w in si.on_wait:
                if str(w.wait_mode) != "sem-ge-imm":
                    continue
                acc_cov[w.ant_name] = max(acc_cov.get(w.ant_name, -1),
                                          w.wait_value)
        cover.append(dict(acc_cov))

    COMPUTE_ENG = {"DVE", "Pool", "Activation", "PE"}
    SKIP_TYPES = {"InstDMACopy", "InstDrain", "InstEventSemaphore", "InstNop",
                  "InstISA", "InstTensorLoad"}

    observed = {}
    for blk in blocks:
        for ins in blk.instructions:
            si = ins.sync_info
            if si is None:
                continue
            eng = eng_of(ins)
            ws = list(si.on_wait)
            if not ws:
                continue
            w_ins = win_of_ins.get(id(ins), -1)
            strip_ok = (eng in COMPUTE_ENG
                        and type(ins).__name__ not in SKIP_TYPES
                        and w_ins >= 0)
            kept = []
            for w in ws:
                if w.ant_name not in nonmono and \
                        str(w.wait_mode) == "sem-ge-imm":
                    if observed.get((eng, w.ant_name), -1) >= w.wait_value:
                        continue
                    if strip_ok and w_ins < len(cover):
                        cov = cover[w_ins].get(w.ant_name, -1)
                        if w.wait_value <= cov:
                            observed[(eng, w.ant_name)] = max(
                                observed.get((eng, w.ant_name), -1),
                                w.wait_value)
                            continue
                kept.append(w)
            if len(kept) > 1:
                self_sem = ENG_SEM.get(eng, "zz")
                non_self = [w for w in kept
                            if not w.ant_name.startswith(self_sem)]
                if non_self:
                    kept = non_self
            if len(kept) > 1 and type(ins).__name__ == "InstMatmult":
                nonpe = [w for w in kept if not w.ant_name.startswith("PE")]
                kept = nonpe if nonpe else kept[:1]
            si.on_wait = kept
            for w in kept:
                if w.ant_name in nonmono:
                    continue
                k = (eng, w.ant_name)
                observed[k] = max(observed.get(k, -1), w.wait_value)

    # stragglers: non-drain instructions with multiple waits drop PE waits
    for blk in blocks:
        for ins in blk.instructions:
            si = ins.sync_info
            if si is None or len(si.on_wait) <= 1:
                continue
            if type(ins).__name__ != "InstDrain":
                nonpe = [w for w in si.on_wait
                         if not w.ant_name.startswith("PE")]
                si.on_wait = nonpe if nonpe else si.on_wait[:1]

    # final out-store: engine waits imply lane ordering
    all_dmas = [ins for blk in blocks for ins in blk.instructions
                if type(ins).__name__ == "InstDMACopy"]
    if all_dmas:
        fin = all_dmas[-1]
        if fin.sync_info and len(fin.sync_info.on_wait) > 1:
            eng = [w for w in fin.sync_info.on_wait
                   if not w.ant_name.startswith(("DMAHW", "DMASW"))]
            if eng:
                fin.sync_info.on_wait = eng

    # tail drain: wait only on the final store's DMA lane
    out_dmas = all_dmas[-1:]
    keep_lanes = set()
    for ins in out_dmas:
        for u in (ins.sync_info.on_update if ins.sync_info else []):
            if u.ant_name.startswith(("DMAHW", "DMASW")):
                keep_lanes.add(u.ant_name)
    for blk in blocks:
        for ins in blk.instructions:
            if type(ins).__name__ == "InstDrain" and ins.sync_info and \
                    len(ins.sync_info.on_wait) > 1:
                lane_ws = [w for w in ins.sync_info.on_wait
                           if w.ant_name in keep_lanes]
                if lane_ws:
                    ins.sync_info.on_wait = lane_ws
    # walrus build can't encode the sem-pool free InstISA; drop it
    for blk in blocks:
        bad = [i for i, ins in enumerate(blk.instructions)
               if type(ins).__name__ == "InstISA"]
        if bad:
            keep = [ins for ins in blk.instructions
                    if type(ins).__name__ != "InstISA"]
            try:
                blk.instructions = keep
            except Exception:
                for i in reversed(bad):
                    del blk.instructions[i]


def _band_mask():
    m = np.zeros((128, NWIN))
    for s in range(SPC):
        for wi, (r0, p0, p1) in enumerate(WINDOWS):
            m[p0:p1, s * NW_PER_SAMPLE + wi] = 1.0
    return m


def _combine(outs):
    """outs: list of [128, NQ*NWIN] f32 arrays (one per core); fp64 combine."""
    A = np.stack([o.reshape(128, NQ, NWIN).astype(np.float64) for o in outs])
    m = _band_mask()[None, :, None, :]
    sums = (A * m).sum(axis=(0, 1, 3))
    s_th, s_ln, s_hn, s_th2 = sums[Q_TH], sums[Q_LN], sums[Q_HN], sums[Q_TH2]
    s_g2p, s_g2d, s_a, s_b = sums[Q_G2P], sums[Q_G2D], sums[Q_SA], sums[Q_SB]
    s_ep, s_tp, s_ab = sums[Q_EP], sums[Q_TP], sums[Q_AB]
    s_thth2, s_thhn = sums[Q_THTH2], sums[Q_THHN]
    mm = m[:, :, 0, :]
    min_g2p = np.where(mm > 0, A[:, :, Q_MINP, :], FBIG).min()
    max_g2p = np.where(mm > 0, A[:, :, Q_MAXP, :], -FBIG).max()
    min_g2d = np.where(mm > 0, A[:, :, Q_MIND, :], FBIG).min()
    max_g2d = np.where(mm > 0, A[:, :, Q_MAXD, :], -FBIG).max()

    # bce: sum softplus = ln2*N - sum ln(1-th)
    s_sp = np.log(2.0) * NTOT - s_ln
    bce1 = (s_sp - s_tp) / NTOT
    bce2 = (s_sp - s_ep) / NTOT

    # gradient magnitude normalization algebra.
    # pred side: a = 0.5*sqrt(g2_th + 4eps)
    e_a2 = s_g2p / (4.0 * NTOT) + EPS
    e_b2 = s_g2d / NTOT + EPS
    amin = 0.5 * np.sqrt(min_g2p + 4.0 * EPS)
    amax = 0.5 * np.sqrt(max_g2p + 4.0 * EPS)
    bmin = np.sqrt(min_g2d + EPS)
    bmax = np.sqrt(max_g2d + EPS)

    def scale_off(lo, hi):
        if hi > lo:
            sc = 1.0 / (hi - lo + EPS)
            return sc, lo * sc
        return 1.0, 0.0

    sa, oa = scale_off(amin, amax)
    sb, ob = scale_off(bmin, bmax)
    cc = oa - ob
    e_a = 0.5 * s_a / NTOT
    e_b = s_b / NTOT
    e_ab = 0.5 * s_ab / NTOT
    grad_cons = (sa * sa * e_a2 + sb * sb * e_b2 + cc * cc
                 - 2.0 * sa * sb * e_ab - 2.0 * cc * sa * e_a
                 + 2.0 * cc * sb * e_b)

    # curvature: cs = (th2+1)/2, pp = (th+1)/2
    sum_ppcs = 0.25 * (s_thth2 + s_th + s_th2 + NTOT)
    curv_cons = -sum_ppcs / NTOT
    # height: pp*hn = ((th+1)/2)*hn
    sum_pphn = 0.5 * (s_thhn + s_hn)
    height_cons = -sum_pphn / NTOT

    geo = grad_cons + 0.5 * height_cons + 0.3 * curv_cons
    total = 0.8 * bce1 + 0.1 * bce2 + 0.1 * geo
    return np.float32(total)


_CONSTS = {}


def kernel(pred, target, dem, _profile=False):
    from concourse.bass_utils import run_bass_kernel_spmd

    if "c" not in _CONSTS:
        _CONSTS["c"] = _build_consts()
    cbf16 = _CONSTS["c"]
    nc = _build_nc()

    p = np.ascontiguousarray(pred.reshape(B, H, W), dtype=np.float32)
    t = np.ascontiguousarray(target.reshape(B, H, W), dtype=np.float32)
    d = np.ascontiguousarray(dem.reshape(B, H, W), dtype=np.float32)

    # host-side per-sample dem stats (fp64): inv = 1/(std+eps), nb = -mean*inv
    d64 = d.reshape(B, -1).astype(np.float64)
    mu = d64.mean(axis=1)
    sd = d64.std(axis=1, ddof=1)
    inv = 1.0 / (sd + EPS)
    nb = -mu * inv

    in_maps = []
    for c in range(NCORES):
        sl = slice(c * SPC, (c + 1) * SPC)
        stats = np.zeros((128, 2 * SPC), np.float32)
        for s in range(SPC):
            stats[:, 2 * s] = inv[c * SPC + s]
            stats[:, 2 * s + 1] = nb[c * SPC + s]
        in_maps.append({
            "pred": p[sl], "target": t[sl], "dem": d[sl],
            "cbf16": cbf16, "stats": stats,
        })
    res = run_bass_kernel_spmd(nc, in_maps, core_ids=list(range(NCORES)),
                               trace=_profile)
    outs = [m["out"] for m in res.results]
    total = _combine(outs)
    if _profile:
        return total, res
    return total


# revision 44
# speedup vs baseline: 1.0109x; 1.0109x over previous
"""Trainium2 Bass kernel for nn_BalancedLoss (composite segmentation loss).

Pure data-parallel over 8 NeuronCores (2 samples each); each core emits a
[128, NQ*NWIN] tile of partial reductions which the host combines in fp64
(including the global min/max normalization algebra for gradient_magnitude).

v2 design vs baseline (915 us):
  - sigmoid replaced by th = tanh(p/2) (activation-table set 0); host algebra
    expands pp = (th+1)/2 through every product sum.
  - softplus(p) = ln2 - ln(1 - th) reuses th; one ACT op (set 5, batched
    per 2 windows with the sqrt block to amortize table loads).
  - curvature sigmoid(10*tanh(0.1*lp)) ~= sigmoid(lp) = (tanh(lp/2)+1)/2:
    one ACT op instead of two (error ~1e-5 of total, tol 2e-2).
  - dem mean/std computed on host (removes the on-device stats prepass).
  - erode threshold fused into the s2 product: (er > 8.5) * p in one DVE op.
  - pred-side Sobel PSUM evacuated by ACT Square (free accum of sum(gx^2));
    dem-side by DVE evac + tensor_tensor_reduce square (free accum).
  - g^2 assembly fused with min/max via tensor_tensor_reduce.
  - engine split so DVE/ACT/Pool all carry ~10 us/window; PE ~9.5.
  - per-window SP observer nop chain: every engine's first touch of the
    window's DMA'd tiles waits one SP sem; stale cross-engine WAR waits are
    stripped in the sync-minimizer using per-window sem-count snapshots.
"""

import os
import numpy as np
from contextlib import ExitStack

DEBUG_NWIN = int(os.environ.get("KDBG_NWIN", "9"))

B, H, W = 16, 1024, 1024
NCORES = 8
SPC = B // NCORES  # samples per core
EPS = 1e-8
NPIX = H * W
NTOT = B * NPIX

WINDOWS = [(0, 0, 125)] + [(122 * w, 3, 125) for w in range(1, 8)] + [(896, 83, 128)]
NW_PER_SAMPLE = len(WINDOWS)
NWIN = SPC * NW_PER_SAMPLE

# accumulator quantity indices, grouped by writer engine:
# ACT block [0..6)
Q_TH, Q_LN, Q_HN, Q_TH2, Q_SA, Q_SB = range(6)
# DVE block [6..17)
(Q_MINP, Q_MAXP, Q_MIND, Q_MAXD, Q_EP, Q_TP, Q_THTH2, Q_THHN,
 Q_AB, Q_G2P, Q_G2D) = range(6, 17)
NQ = 17
NQ_ACT, NQ_DVE = 6, 11

FBIG = 3.0e38


def _tridiag(a, b, c, dtype):
    # out[p] = a*x[p-1] + b*x[p] + c*x[p+1] for matmul out = lhsT.T @ x
    M = np.zeros((128, 128), dtype=np.float64)
    idx = np.arange(128)
    M[idx, idx] = b
    M[idx[:-1], idx[1:]] = a
    M[idx[1:], idx[:-1]] = c
    return M.astype(dtype)


def _build_consts():
    import ml_dtypes
    bf16 = ml_dtypes.bfloat16
    mats = [
        _tridiag(1, 1, 1, bf16),     # 0 M111   (box sum / dilate / erode)
        _tridiag(1, 2, 1, bf16),     # 1 M121   (sobel x, +1 col)
        _tridiag(-1, -2, -1, bf16),  # 2 -M121  (sobel x, -1 col)
        _tridiag(-1, 0, 1, bf16),    # 3 Mm101  (sobel y, +-1 col)
        _tridiag(-2, 0, 2, bf16),    # 4 Mm202  (sobel y, center)
        _tridiag(1, -8, 1, bf16),    # 5 M1m81  (9*(box_mean - x) center tap)
        _tridiag(0, 1, 0, bf16),     # 6 I      (laplacian +-1 col)
        _tridiag(1, -4, 1, bf16),    # 7 M1m41  (laplacian center)
        np.zeros((128, 2), bf16),    # 8 zero pad source
    ]
    return np.concatenate(mats, axis=1)  # [128, 8*128+2]


_NC_CACHE = {}


def _build_nc():
    if "nc" in _NC_CACHE:
        return _NC_CACHE["nc"]
    import concourse.bass as bass
    import concourse.tile as tile
    from concourse import mybir

    fp32 = mybir.dt.float32
    bf16 = mybir.dt.bfloat16
    ALU = mybir.AluOpType
    ACTF = mybir.ActivationFunctionType

    nc = bass.Bass("TRN2", target_bir_lowering=False)
    pred_d = nc.declare_dram_parameter("pred", [SPC, H, W], fp32, isOutput=False)
    targ_d = nc.declare_dram_parameter("target", [SPC, H, W], fp32, isOutput=False)
    dem_d = nc.declare_dram_parameter("dem", [SPC, H, W], fp32, isOutput=False)
    cbf16_d = nc.declare_dram_parameter("cbf16", [128, 8 * 128 + 2], bf16,
                                        isOutput=False)
    stats_d = nc.declare_dram_parameter("stats", [128, 2 * SPC], fp32,
                                        isOutput=False)
    out_d = nc.declare_dram_parameter("out", [128, NQ * NWIN], fp32, isOutput=True)

    # instruction markers for the window-snapshot strip rule
    win_marks = []   # per gw: dict(absorb=ins, obs=ins)
    strip_info = {"marks": win_marks}

    with tile.TileContext(nc) as tc:
        ctx = ExitStack()
        const = ctx.enter_context(tc.tile_pool(name="const", bufs=1))
        accp = ctx.enter_context(tc.tile_pool(name="accp", bufs=1))
        scr = ctx.enter_context(tc.tile_pool(name="scr", bufs=2))
        dfr = ctx.enter_context(tc.tile_pool(name="dfr", bufs=4))
        psum_v = ctx.enter_context(tc.tile_pool(name="psum_v", bufs=2, space="PSUM"))
        psum_a = ctx.enter_context(tc.tile_pool(name="psum_a", bufs=2, space="PSUM"))

        CB = const.tile([128, 8 * 128 + 2], bf16)

        def dma2(out_ap, in_ap, after=None):
            a = nc.sync.dma_start(out=out_ap[0:64], in_=in_ap[0:64])
            b = nc.sync.dma_start(out=out_ap[64:128], in_=in_ap[64:128])
            if after is not None:
                tile.add_dep_helper(a.ins, after.ins, sync=False,
                                    reason="order after absorber")
                tile.add_dep_helper(b.ins, after.ins, sync=False,
                                    reason="order after absorber")
            return [a, b]

        startup_dmas = []
        startup_dmas += dma2(CB, cbf16_d[:, :])
        ST = const.tile([128, 2 * SPC], fp32)
        startup_dmas.append(nc.sync.dma_start(out=ST, in_=stats_d[:, :]))
        ZPAD = cbf16_d[:, 8 * 128:8 * 128 + 2]

        def mb(i):
            return CB[:, i * 128:(i + 1) * 128]

        M111B, M121B, M121NB, M101B, M202B, M1M81B, IB, MLAPB = (
            mb(0), mb(1), mb(2), mb(3), mb(4), mb(5), mb(6), mb(7))

        # persistent rotating tiles (Tpp 4-deep for 4-window ACT batching)
        DBL = {}
        for par in (0, 1):
            DBL[("Tt", par)] = const.tile([128, 1024], fp32, name=f"Tt{par}")
            DBL[("Tp", par)] = const.tile([128, 1024], fp32, name=f"Tp{par}")
            DBL[("Td", par)] = const.tile([128, 1024], fp32, name=f"Td{par}")
            for nm in ("Ttb", "Tdb", "Te", "Tdl"):
                DBL[(nm, par)] = const.tile([128, 1026], bf16, name=f"{nm}{par}")
        for par in range(4):
            DBL[("Tpp", par)] = const.tile([128, 1026], bf16, name=f"Tpp{par}")

        # zero the pad columns of all padded tiles via DMA from the zero
        # columns of the const tensor (bf16 memset is not encodable)
        for par in (0, 1):
            for nm in ("Ttb", "Tdb", "Te", "Tdl"):
                t = DBL[(nm, par)]
                startup_dmas += dma2(t[:, 0:1026:1025], ZPAD)
        for par in range(4):
            startup_dmas += dma2(DBL[("Tpp", par)][:, 0:1026:1025], ZPAD)
        # eps bias tiles (fp32 memsets encode fine)
        EPS4B = const.tile([128, 1], fp32)
        nc.gpsimd.memset(EPS4B, 4.0 * EPS)
        EPS1B = const.tile([128, 1], fp32)
        eps_memset = nc.gpsimd.memset(EPS1B, EPS)
        # windows 0/1's obs chains wait all startup DMAs + the eps memsets,
        # so the cover-based strip handles every startup dependency.
        startup_last = startup_dmas + [eps_memset]

        # accumulators: ACT gets a side tile; DVE block lives in ACCBIG
        ACCBIG = accp.tile([128, NQ * NWIN], fp32, name="accbig")
        ACTACC = accp.tile([128, NQ_ACT * NWIN], fp32, name="actacc")

        def acc(q, wcol):
            if q < NQ_ACT:
                return ACTACC[:, q * NWIN + wcol:q * NWIN + wcol + 1]
            return ACCBIG[:, q * NWIN + wcol:q * NWIN + wcol + 1]

        def _all_instrs():
            return [i for b in nc.m.functions[0].blocks for i in b.instructions]

        def conv(ps, groups, srctile, touch=False):
            # tiny touch matmuls absorb the psum-slot WAR wait when the
            # producer and the stale WAR reader live on different engines
            # (1-wait limit); elsewhere the two waits share a sem and merge.
            if touch:
                for c0 in (0, 512):
                    nc.tensor.matmul(ps[0:1, c0:c0 + 1], CB[:, 0:1],
                                     CB[:, 0:1], start=True, stop=True)
            last = None
            for c0 in (0, 512):
                for i, (mat, dx) in enumerate(groups):
                    last = nc.tensor.matmul(
                        ps[:, c0:c0 + 512], mat,
                        srctile[:, c0 + dx + 1:c0 + dx + 1 + 512],
                        start=(i == 0), stop=(i == len(groups) - 1))
            return last

        last_eng_op = {}   # gw -> {engine: ins} last op per engine per window
        deferred_act = []  # (kind, gw) ACT sqrt/ln ops batched per 2 windows

        nwin_run = SPC * min(DEBUG_NWIN, NW_PER_SAMPLE)
        for s in range(SPC):
            inv_ap = ST[:, 2 * s:2 * s + 1]
            nb_ap = ST[:, 2 * s + 1:2 * s + 2]
            for wi, (r0, p0, p1) in enumerate(WINDOWS[:DEBUG_NWIN]):
                gw = s * min(DEBUG_NWIN, NW_PER_SAMPLE) + wi
                wcol = s * NW_PER_SAMPLE + wi
                par = gw % 2
                Tt, Tp, Td = DBL[("Tt", par)], DBL[("Tp", par)], DBL[("Td", par)]
                Ttb, Tdb = DBL[("Ttb", par)], DBL[("Tdb", par)]
                Tpp = DBL[("Tpp", gw % 4)]
                Te, Tdl = DBL[("Te", par)], DBL[("Tdl", par)]

                win_i0 = len(_all_instrs())
                # ---- absorber: wait last op of each engine two windows ago.
                # NoOps have a single wait slot; chain 1-wait nops — the
                # in-order SP sequencer serializes the chain.
                prev = last_eng_op.get(gw - 2, {}) if gw >= 2 else {}
                absorb = None
                chain = None
                chain_ins = []
                for eng, ins in list(prev.items()) or [(None, None)]:
                    o = nc.sync.nop()
                    if ins is not None:
                        tile.add_dep_helper(o.ins, ins.ins, sync=True,
                                            reason=f"absorb w-2 {eng}")
                    if chain is not None:
                        tile.add_dep_helper(o.ins, chain.ins, sync=False,
                                            reason="absorb chain")
                    if absorb is None:
                        absorb = o
                    chain = o
                    chain_ins.append(o.ins)
                dmas = []
                dmas += dma2(Tt, targ_d[s, r0:r0 + 128, :], after=chain)
                dmas += dma2(Tp, pred_d[s, r0:r0 + 128, :], after=chain)
                dmas += dma2(Td, dem_d[s, r0:r0 + 128, :], after=chain)
                obs = None
                obs_deps = list(dmas)
                if gw < 2:
                    obs_deps += startup_last
                for dd in obs_deps:
                    o = nc.sync.nop()
                    tile.add_dep_helper(o.ins, dd.ins, sync=True,
                                        reason="window dma observer")
                    if obs is not None:
                        tile.add_dep_helper(o.ins, obs.ins, sync=False,
                                            reason="obs chain")
                    obs = o
                    chain_ins.append(o.ins)
                win_marks.append({"absorb": absorb.ins, "obs": obs.ins,
                                  "chain": chain_ins})
                leo = {}

                # ---- DVE: target convert first (gates PE bx)
                tb_i = nc.vector.tensor_scalar(
                    out=Ttb[:, 1:1025], in0=Tt, scalar1=1.0, scalar2=None,
                    op0=ALU.mult)
                tile.add_dep_helper(tb_i.ins, obs.ins, sync=True, reason="obs")

                # ---- ACT: th gates PE sobel-p; dem convert gates sobel-d
                th_i = nc.scalar.activation(
                    out=Tpp[:, 1:1025], in_=Tp, func=ACTF.Tanh, scale=0.5,
                    accum_out=acc(Q_TH, wcol))
                tile.add_dep_helper(th_i.ins, obs.ins, sync=True, reason="obs")
                if gw >= 1:
                    tile.add_dep_helper(th_i.ins,
                                        last_eng_op[gw - 1]["ACT"].ins,
                                        sync=False, reason="act order")
                db_i = nc.scalar.activation(
                    out=Tdb[:, 1:1025], in_=Td, func=ACTF.Copy)
                tile.add_dep_helper(db_i.ins, obs.ins, sync=True, reason="obs")

                # ---- PE: bx (edge detect of target)
                bx = psum_v.tile([128, 1024], fp32, tag="psv")
                conv(bx, [(M111B, -1), (M1M81B, 0), (M111B, 1)], Ttb,
                     touch=True)

                # ---- PE: sobel on pred prob (after th)
                gxp = psum_a.tile([128, 1024], fp32, tag="psa")
                conv(gxp, [(M121NB, -1), (M121B, 1)], Tpp, touch=True)
                gyp = psum_a.tile([128, 1024], fp32, tag="psa")
                conv(gyp, [(M101B, -1), (M101B, 1), (M202B, 0)], Tpp)

                # ---- ACT+DVE: edge threshold (gates PE dl)
                # |bx| > 1.35 as bx^2 > 1.8225 (Square is a set-0 ACT op)
                bx2 = scr.tile([128, 1024], bf16, tag="bx2")
                nc.scalar.activation(out=bx2, in_=bx, func=ACTF.Square)
                e_i = nc.vector.tensor_scalar(
                    out=Te[:, 1:1025], in0=bx2, scalar1=1.8225, scalar2=None,
                    op0=ALU.is_gt)

                # ---- ACT: pred-side square evacuations (set0, accum)
                sq_px = scr.tile([128, 1024], bf16, tag="sq_px")
                nc.scalar.activation(out=sq_px, in_=gxp, func=ACTF.Square)
                sq_py = scr.tile([128, 1024], bf16, tag="sq_py")
                nc.scalar.activation(out=sq_py, in_=gyp, func=ACTF.Square)

                # ---- PE: dilate
                dl = psum_v.tile([128, 1024], fp32, tag="psv")
                conv(dl, [(M111B, -1), (M111B, 0), (M111B, 1)], Te, touch=True)

                # ---- PE: sobel on dem
                gxd = psum_a.tile([128, 1024], fp32, tag="psa")
                conv(gxd, [(M121NB, -1), (M121B, 1)], Tdb)
                gyd = psum_a.tile([128, 1024], fp32, tag="psa")
                conv(gyd, [(M101B, -1), (M101B, 1), (M202B, 0)], Tdb)

                # ---- DVE: dilate threshold (gates PE er)
                nc.vector.tensor_scalar(
                    out=Tdl[:, 1:1025], in0=dl, scalar1=0.5, scalar2=None,
                    op0=ALU.is_gt)

                # ---- ACT: dem-side square evacuations (set0, accum)
                sq_dx = scr.tile([128, 1024], bf16, tag="sq_dx")
                nc.scalar.activation(out=sq_dx, in_=gxd, func=ACTF.Square)
                sq_dy = scr.tile([128, 1024], bf16, tag="sq_dy")
                nc.scalar.activation(out=sq_dy, in_=gyd, func=ACTF.Square)

                # ---- PE: erode
                er = psum_v.tile([128, 1024], fp32, tag="psv")
                conv(er, [(M111B, -1), (M111B, 0), (M111B, 1)], Tdl,
                     touch=True)

                # ---- DVE: g2 assembly (STT carries the sum) + min/max
                g2p = dfr.tile([128, 1024], bf16, tag="g2p")
                nc.vector.scalar_tensor_tensor(
                    out=g2p, in0=sq_px, scalar=1.0, in1=sq_py,
                    op0=ALU.mult, op1=ALU.add, accum_out=acc(Q_G2P, wcol))
                nc.vector.tensor_reduce(out=acc(Q_MINP, wcol), in_=g2p,
                                        axis=mybir.AxisListType.X, op=ALU.min)
                nc.vector.tensor_reduce(out=acc(Q_MAXP, wcol), in_=g2p,
                                        axis=mybir.AxisListType.X, op=ALU.max)
                g2d = dfr.tile([128, 1024], bf16, tag="g2d")
                nc.vector.scalar_tensor_tensor(
                    out=g2d, in0=sq_dx, scalar=1.0, in1=sq_dy,
                    op0=ALU.mult, op1=ALU.add, accum_out=acc(Q_G2D, wcol))
                nc.vector.tensor_reduce(out=acc(Q_MIND, wcol), in_=g2d,
                                        axis=mybir.AxisListType.X, op=ALU.min)
                nc.vector.tensor_reduce(out=acc(Q_MAXD, wcol), in_=g2d,
                                        axis=mybir.AxisListType.X, op=ALU.max)

                # ---- PE: laplacian
                lp = psum_v.tile([128, 1024], fp32, tag="psv")
                pe_last = conv(lp, [(IB, -1), (MLAPB, 0), (IB, 1)], Tdb,
                               touch=True)

                # ---- DVE: fused erode-threshold * pred accumulation
                jp = scr.tile([128, 1024], bf16, tag="junkD")
                s2_i = nc.vector.scalar_tensor_tensor(
                    out=jp, in0=er, scalar=8.5, in1=Tp,
                    op0=ALU.is_gt, op1=ALU.mult, accum_out=acc(Q_EP, wcol))

                # ---- ACT: z-score square + gaussian + curvature tanh
                z2 = scr.tile([128, 1024], bf16, tag="z2")
                nc.scalar.activation(out=z2, in_=Tdb[:, 1:1025],
                                     func=ACTF.Square, scale=inv_ap, bias=nb_ap)
                hn = scr.tile([128, 1024], bf16, tag="hn")
                nc.scalar.activation(out=hn, in_=z2, func=ACTF.Exp, scale=-0.5,
                                     accum_out=acc(Q_HN, wcol))
                cs = scr.tile([128, 1024], bf16, tag="cs")
                cs_i = nc.scalar.activation(out=cs, in_=lp, func=ACTF.Tanh,
                                            scale=0.5,
                                            accum_out=acc(Q_TH2, wcol))

                # ---- DVE: product sums
                j1 = scr.tile([128, 1024], bf16, tag="junkP")
                s1_i = nc.vector.scalar_tensor_tensor(
                    out=j1, in0=Ttb[:, 1:1025], scalar=1.0, in1=Tp,
                    op0=ALU.mult, op1=ALU.mult, accum_out=acc(Q_TP, wcol))
                j4 = scr.tile([128, 1024], bf16, tag="junkP")
                nc.vector.scalar_tensor_tensor(
                    out=j4, in0=Tpp[:, 1:1025], scalar=1.0, in1=cs,
                    op0=ALU.mult, op1=ALU.mult, accum_out=acc(Q_THTH2, wcol))
                j5 = scr.tile([128, 1024], bf16, tag="junkP")
                s5_i = nc.vector.scalar_tensor_tensor(
                    out=j5, in0=Tpp[:, 1:1025], scalar=1.0, in1=hn,
                    op0=ALU.mult, op1=ALU.mult, accum_out=acc(Q_THHN, wcol))

                # ---- deferred ACT blocks (sqrt set3, ln set5) every 2 windows
                deferred_act.append((gw, wcol, g2p, g2d, gw % 4))
                act_last = cs_i
                dve_extra = None
                if gw % 4 == 3 or gw == nwin_run - 1:
                    av_tiles = []
                    prev_a = cs_i
                    for (dgw, dwcol, dg2p, dg2d, dpar) in deferred_act:
                        av_p = dfr.tile([128, 1024], bf16, tag="av_p",
                                        name=f"av_p{dgw}")
                        a1 = nc.scalar.activation(out=av_p, in_=dg2p,
                                                  func=ACTF.Sqrt, bias=EPS4B,
                                                  accum_out=acc(Q_SA, dwcol))
                        tile.add_dep_helper(a1.ins, prev_a.ins, sync=False,
                                            reason="act order")
                        av_d = dfr.tile([128, 1024], bf16, tag="av_d",
                                        name=f"av_d{dgw}")
                        a2 = nc.scalar.activation(out=av_d, in_=dg2d,
                                                  func=ACTF.Sqrt, bias=EPS1B,
                                                  accum_out=acc(Q_SB, dwcol))
                        tile.add_dep_helper(a2.ins, a1.ins, sync=False,
                                            reason="act order")
                        prev_a = a2
                        av_tiles.append((dgw, dwcol, dpar, av_p, av_d))
                    for (dgw, dwcol, dpar, av_p, av_d) in av_tiles:
                        jl = dfr.tile([128, 1024], bf16, tag="junkA",
                                      name=f"jl{dgw}")
                        act_last = nc.scalar.activation(
                            out=jl, in_=DBL[("Tpp", dpar)][:, 1:1025],
                            func=ACTF.Ln, scale=-1.0, bias=1.0,
                            accum_out=acc(Q_LN, dwcol))
                        tile.add_dep_helper(act_last.ins, prev_a.ins,
                                            sync=False, reason="act order")
                        prev_a = act_last
                    # DVE: s3 = sum(av_p*av_d)
                    for (dgw, dwcol, dpar, av_p, av_d) in av_tiles:
                        j3 = dfr.tile([128, 1024], bf16, tag="junkQ",
                                      name=f"j3{dgw}")
                        dve_extra = nc.vector.scalar_tensor_tensor(
                            out=j3, in0=av_p, scalar=1.0, in1=av_d,
                            op0=ALU.mult, op1=ALU.mult,
                            accum_out=acc(Q_AB, dwcol))
                    deferred_act = []

                # last ops per engine for the absorber two windows ahead.
                # The scheduler may reorder within an engine stream, so pin
                # each designated last op behind all its window peers with
                # nosync (ordering-only) edges.
                leo["PE"] = pe_last
                leo["DVE"] = dve_extra if dve_extra is not None else s5_i
                leo["ACT"] = act_last
                new_ins = _all_instrs()[win_i0:]
                for eng_key, lo in leo.items():
                    sem = {"PE": "PE", "DVE": "DVE", "ACT": "Activation"}[eng_key]
                    for x in new_ins:
                        e = getattr(x, "engine", None)
                        if e is None or str(e).split(".")[-1] != sem:
                            continue
                        if x is lo.ins:
                            continue
                        tile.add_dep_helper(lo.ins, x, sync=False,
                                            reason="fence order")
                last_eng_op[gw] = leo

        # tail: mirror the ACT accum tile into ACCBIG via DVE, then store
        nc.vector.tensor_scalar(out=ACCBIG[:, 0:NQ_ACT * NWIN], in0=ACTACC,
                                scalar1=1.0, scalar2=None, op0=ALU.mult)
        nc.sync.dma_start(out=out_d[:, :], in_=ACCBIG[:, :])
        ctx.close()

    _minimize_syncs(nc, strip_info)
    _NC_CACHE["nc"] = nc
    return nc


def _minimize_syncs(nc, strip_info):
    """Walrus codegen wants few sync-waits per instruction. Tile's emission is
    per-instruction structural; prune:
      1. per-engine observed-threshold replay (monotone sems).
      2. window-snapshot strip: ops of window w may drop waits with
         thresholds <= the sem counts at the end of window w-2 (covered by
         the absorber -> dma -> obs chain) and DMA-queue waits <= counts at
         obs_w. Applies to compute engines only.
      3. same-engine self-sem waits (program order).
      4. PE pc-monotone cleanup, final-store and drain special cases.
    """
    marks = strip_info["marks"]
    absorb_ids = {id(m["absorb"]): k for k, m in enumerate(marks)}
    obs_ids = {id(m["obs"]): k for k, m in enumerate(marks)}

    ENG_SEM = {"PE": "PE", "DVE": "DVE", "Activation": "Activation",
               "Pool": "Pool", "SP": "SP_sequencer"}

    def eng_of(ins):
        e = getattr(ins, "engine", None)
        return str(e).split(".")[-1] if e is not None else "SP"

    blocks = nc.m.functions[0].blocks
    nonmono = set()
    for blk in blocks:
        for ins in blk.instructions:
            if ins.sync_info is None:
                continue
            for u in ins.sync_info.on_update:
                um = str(u.update_mode)
                if "sub" in um or "dec" in um:
                    nonmono.add(u.ant_name)

    # pass A: window attribution in EMISSION order (instruction-id order —
    # the scheduler reorders the block), and per-window wait coverage read
    # directly off the absorber/observer chain nops' framework-assigned
    # wait values (same value space as every other wait).
    def _ord(ins):
        nm = getattr(ins, "name", "")
        try:
            return int(str(nm).rsplit("-", 1)[-1])
        except ValueError:
            return 1 << 60
    all_ins = [ins for blk in blocks for ins in blk.instructions]
    all_ins.sort(key=_ord)
    cur_win = -1
    win_of_ins = {}
    for ins in all_ins:
        iid = id(ins)
        win_of_ins[iid] = cur_win
        if iid in obs_ids:
            k = obs_ids[iid]
            cur_win = k
            win_of_ins[iid] = k
    # coverage: cover[k][sem] = max wait value guaranteed satisfied once
    # window k's obs chain has retired (accumulates across windows via the
    # in-order SP sequencer).
    cover = []
    acc_cov = {}
    for k, m in enumerate(marks):
        for cins in m["chain"]:
            si = cins.sync_info
            if si is None:
                continue
            for w in si.on_wait:
                if str(w.wait_mode) != "sem-ge-imm":
                    continue
                acc_cov[w.ant_name] = max(acc_cov.get(w.ant_name, -1),
                                          w.wait_value)
        cover.append(dict(acc_cov))

    COMPUTE_ENG = {"DVE", "Pool", "Activation", "PE"}
    SKIP_TYPES = {"InstDMACopy", "InstDrain", "InstEventSemaphore", "InstNop",
                  "InstISA", "InstTensorLoad"}

    observed = {}
    for blk in blocks:
        for ins in blk.instructions:
            si = ins.sync_info
            if si is None:
                continue
            eng = eng_of(ins)
            ws = list(si.on_wait)
            if not ws:
                continue
            w_ins = win_of_ins.get(id(ins), -1)
            strip_ok = (eng in COMPUTE_ENG
                        and type(ins).__name__ not in SKIP_TYPES
                        and w_ins >= 0)
            kept = []
            for w in ws:
                if w.ant_name not in nonmono and \
                        str(w.wait_mode) == "sem-ge-imm":
                    if observed.get((eng, w.ant_name), -1) >= w.wait_value:
                        continue
                    if strip_ok and w_ins < len(cover):
                        cov = cover[w_ins].get(w.ant_name, -1)
                        if w.wait_value <= cov:
                            observed[(eng, w.ant_name)] = max(
                                observed.get((eng, w.ant_name), -1),
                                w.wait_value)
                            continue
                kept.append(w)
            if len(kept) > 1:
                self_sem = ENG_SEM.get(eng, "zz")
                non_self = [w for w in kept
                            if not w.ant_name.startswith(self_sem)]
                if non_self:
                    kept = non_self
            if len(kept) > 1 and type(ins).__name__ == "InstMatmult":
                nonpe = [w for w in kept if not w.ant_name.startswith("PE")]
                kept = nonpe if nonpe else kept[:1]
            si.on_wait = kept
            for w in kept:
                if w.ant_name in nonmono:
                    continue
                k = (eng, w.ant_name)
                observed[k] = max(observed.get(k, -1), w.wait_value)

    # stragglers: non-drain instructions with multiple waits drop PE waits
    for blk in blocks:
        for ins in blk.instructions:
            si = ins.sync_info
            if si is None or len(si.on_wait) <= 1:
                continue
            if type(ins).__name__ != "InstDrain":
                nonpe = [w for w in si.on_wait
                         if not w.ant_name.startswith("PE")]
                si.on_wait = nonpe if nonpe else si.on_wait[:1]

    # final out-store: engine waits imply lane ordering
    all_dmas = [ins for blk in blocks for ins in blk.instructions
                if type(ins).__name__ == "InstDMACopy"]
    if all_dmas:
        fin = all_dmas[-1]
        if fin.sync_info and len(fin.sync_info.on_wait) > 1:
            eng = [w for w in fin.sync_info.on_wait
                   if not w.ant_name.startswith(("DMAHW", "DMASW"))]
            if eng:
                fin.sync_info.on_wait = eng

    # tail drain: wait only on the final store's DMA lane
    out_dmas = all_dmas[-1:]
    keep_lanes = set()
    for ins in out_dmas:
        for u in (ins.sync_info.on_update if ins.sync_info else []):
            if u.ant_name.startswith(("DMAHW", "DMASW")):
                keep_lanes.add(u.ant_name)
    for blk in blocks:
        for ins in blk.instructions:
            if type(ins).__name__ == "InstDrain" and ins.sync_info and \
                    len(ins.sync_info.on_wait) > 1:
                lane_ws = [w for w in ins.sync_info.on_wait
                           if w.ant_name in keep_lanes]
                if lane_ws:
                    ins.sync_info.on_wait = lane_ws
    # walrus build can't encode the sem-pool free InstISA; drop it
    for blk in blocks:
        bad = [i for i, ins in enumerate(blk.instructions)
               if type(ins).__name__ == "InstISA"]
        if bad:
            keep = [ins for ins in blk.instructions
                    if type(ins).__name__ != "InstISA"]
            try:
                blk.instructions = keep
            except Exception:
                for i in reversed(bad):
                    del blk.instructions[i]


def _band_mask():
    m = np.zeros((128, NWIN))
    for s in range(SPC):
        for wi, (r0, p0, p1) in enumerate(WINDOWS):
            m[p0:p1, s * NW_PER_SAMPLE + wi] = 1.0
    return m


def _combine(outs):
    """outs: list of [128, NQ*NWIN] f32 arrays (one per core); fp64 combine."""
    A = np.stack([o.reshape(128, NQ, NWIN).astype(np.float64) for o in outs])
    m = _band_mask()[None, :, None, :]
    sums = (A * m).sum(axis=(0, 1, 3))
    s_th, s_ln, s_hn, s_th2 = sums[Q_TH], sums[Q_LN], sums[Q_HN], sums[Q_TH2]
    s_g2p, s_g2d, s_a, s_b = sums[Q_G2P], sums[Q_G2D], sums[Q_SA], sums[Q_SB]
    s_ep, s_tp, s_ab = sums[Q_EP], sums[Q_TP], sums[Q_AB]
    s_thth2, s_thhn = sums[Q_THTH2], sums[Q_THHN]
    mm = m[:, :, 0, :]
    min_g2p = np.where(mm > 0, A[:, :, Q_MINP, :], FBIG).min()
    max_g2p = np.where(mm > 0, A[:, :, Q_MAXP, :], -FBIG).max()
    min_g2d = np.where(mm > 0, A[:, :, Q_MIND, :], FBIG).min()
    max_g2d = np.where(mm > 0, A[:, :, Q_MAXD, :], -FBIG).max()

    # bce: sum softplus = ln2*N - sum ln(1-th)
    s_sp = np.log(2.0) * NTOT - s_ln
    bce1 = (s_sp - s_tp) / NTOT
    bce2 = (s_sp - s_ep) / NTOT

    # gradient magnitude normalization algebra.
    # pred side: a = 0.5*sqrt(g2_th + 4eps)
    e_a2 = s_g2p / (4.0 * NTOT) + EPS
    e_b2 = s_g2d / NTOT + EPS
    amin = 0.5 * np.sqrt(min_g2p + 4.0 * EPS)
    amax = 0.5 * np.sqrt(max_g2p + 4.0 * EPS)
    bmin = np.sqrt(min_g2d + EPS)
    bmax = np.sqrt(max_g2d + EPS)

    def scale_off(lo, hi):
        if hi > lo:
            sc = 1.0 / (hi - lo + EPS)
            return sc, lo * sc
        return 1.0, 0.0

    sa, oa = scale_off(amin, amax)
    sb, ob = scale_off(bmin, bmax)
    cc = oa - ob
    e_a = 0.5 * s_a / NTOT
    e_b = s_b / NTOT
    e_ab = 0.5 * s_ab / NTOT
    grad_cons = (sa * sa * e_a2 + sb * sb * e_b2 + cc * cc
                 - 2.0 * sa * sb * e_ab - 2.0 * cc * sa * e_a
                 + 2.0 * cc * sb * e_b)

    # curvature: cs = (th2+1)/2, pp = (th+1)/2
    sum_ppcs = 0.25 * (s_thth2 + s_th + s_th2 + NTOT)
    curv_cons = -sum_ppcs / NTOT
    # height: pp*hn = ((th+1)/2)*hn
    sum_pphn = 0.5 * (s_thhn + s_hn)
    height_cons = -sum_pphn / NTOT

    geo = grad_cons + 0.5 * height_cons + 0.3 * curv_cons
    total = 0.8 * bce1 + 0.1 * bce2 + 0.1 * geo
    return np.float32(total)


_CONSTS = {}


def kernel(pred, target, dem, _profile=False):
    from concourse.bass_utils import run_bass_kernel_spmd

    if "c" not in _CONSTS:
        _CONSTS["c"] = _build_consts()
    cbf16 = _CONSTS["c"]
    nc = _build_nc()

    p = np.ascontiguousarray(pred.reshape(B, H, W), dtype=np.float32)
    t = np.ascontiguousarray(target.reshape(B, H, W), dtype=np.float32)
    d = np.ascontiguousarray(dem.reshape(B, H, W), dtype=np.float32)

    # host-side per-sample dem stats (fp64): inv = 1/(std+eps), nb = -mean*inv
    d64 = d.reshape(B, -1).astype(np.float64)
    mu = d64.mean(axis=1)
    sd = d64.std(axis=1, ddof=1)
    inv = 1.0 / (sd + EPS)
    nb = -mu * inv

    in_maps = []
    for c in range(NCORES):
        sl = slice(c * SPC, (c + 1) * SPC)
        stats = np.zeros((128, 2 * SPC), np.float32)
        for s in range(SPC):
            stats[:, 2 * s] = inv[c * SPC + s]
            stats[:, 2 * s + 1] = nb[c * SPC + s]
        in_maps.append({
            "pred": p[sl], "target": t[sl], "dem": d[sl],
            "cbf16": cbf16, "stats": stats,
        })
    res = run_bass_kernel_spmd(nc, in_maps, core_ids=list(range(NCORES)),
                               trace=_profile)
    outs = [m["out"] for m in res.results]
    total = _combine(outs)
    if _profile:
        return total, res
    return total
